# revision 7
# baseline (speedup 1.0000x reference)
"""LocationSensitiveAttention Trainium2 kernel.

Full (unsharded) inputs in, full outputs out. Internally: data-parallel over
the batch axis across 8 NeuronCores (16 batch elements per core); all params
replicated.

Per-core device algorithm (T=1024, B_loc=16, H=512, A=128, K=32, F=31):
  pk^T[a, (b,t)]  = sum_h Wk[h,a] * enc[t,b,h]      (PE, Wk stationary, enc^T tiles)
  pl^T[a, (b,t)]  = sum_f U[f,a] * pa_pad[b, t+f]   (PE, U = filt^T @ W_loc, im2col via
                                                     shifted-replica sliding-window AP)
  S = tanh(pk+pl+pq[b])                             (ACT, pq as per-partition bias)
  score = v . S                                     (PE, tanh-tile stationary, v moving)
  alignment = exp(score)/Z                          (ACT exp + ones-matmul column sums)
  context[b,:] = sum_t alignment * enc              (PE, alignment column stationary)
"""

import numpy as np
import ml_dtypes

import concourse.bass as bass
import concourse.tile as tile
from concourse import bacc, mybir
from concourse.bass_utils import run_bass_kernel_spmd

BF16 = mybir.dt.bfloat16
F32 = mybir.dt.float32
AF = mybir.ActivationFunctionType

T, B, H = 1024, 128, 512
A, K, F = 128, 32, 31
NCORES = 8
BL = B // NCORES          # 16 batch elements per core
PAD = (F - 1) // 2        # 15
ROW = T + 2 * PAD         # 1054: padded per-batch alignment row
PA_LEN = BL * ROW + 32    # shifted-window source length (16896)
TH = T // 512             # 2 t-halves per batch element
NB = 512 // 128           # 4 128-blocks per t-half


def _build_program() -> bacc.Bacc:
    nc = bacc.Bacc("TRN2", target_bir_lowering=False, debug=False)

    encT = nc.dram_tensor("encT", [BL, H, T], BF16, kind="ExternalInput").ap()
    encN = nc.dram_tensor("encN", [BL, T, H], BF16, kind="ExternalInput").ap()
    paT = nc.dram_tensor("paT", [PA_LEN], BF16, kind="ExternalInput").ap()
    qT = nc.dram_tensor("qT", [H, BL], F32, kind="ExternalInput").ap()
    wk = nc.dram_tensor("wk", [H, A], BF16, kind="ExternalInput").ap()
    wq = nc.dram_tensor("wq", [H, A], F32, kind="ExternalInput").ap()
    wloc = nc.dram_tensor("wloc", [K, A], F32, kind="ExternalInput").ap()
    filt = nc.dram_tensor("filt", [K, F], F32, kind="ExternalInput").ap()
    vv = nc.dram_tensor("vv", [A, 1], BF16, kind="ExternalInput").ap()
    ctx_out = nc.dram_tensor("ctx_out", [BL, H], F32, kind="ExternalOutput").ap()
    al_out = nc.dram_tensor("al_out", [T, BL], F32, kind="ExternalOutput").ap()

    HK = H // 128  # 4 contraction chunks

    with tile.TileContext(nc) as tc:
        from contextlib import ExitStack

        with ExitStack() as ctx:
            cp = ctx.enter_context(tc.tile_pool(name="consts", bufs=1))

            # --- replicated params into SBUF ---
            wk_sb = cp.tile([128, HK, A], BF16)
            nc.sync.dma_start(wk_sb[:], wk.rearrange("(hk p) a -> p hk a", p=128))
            wq_sb = cp.tile([128, HK, A], F32)
            nc.sync.dma_start(wq_sb[:], wq.rearrange("(hk p) a -> p hk a", p=128))
            qT_sb = cp.tile([128, HK, BL], F32)
            nc.sync.dma_start(qT_sb[:], qT.rearrange("(hk p) b -> p hk b", p=128))
            filt_sb = cp.tile([K, F], F32)
            nc.sync.dma_start(filt_sb[:], filt)
            wloc_sb = cp.tile([K, A], F32)
            nc.sync.dma_start(wloc_sb[:], wloc)
            vv_sb = cp.tile([A, 1], BF16)
            nc.sync.dma_start(vv_sb[:], vv)
            # padded previous_alignment, replicated with +1-element shift per
            # partition: pa_rep[f, x] = paT[x + f] -> sliding conv windows are
            # plain [31, 512] slices.
            pa_rep = cp.tile([F, BL * ROW], BF16)
            pa_src = bass.AP(paT.tensor, 0, [[1, F], [1, BL * ROW]])
            nc.sync.dma_start(pa_rep[:], pa_src)

            ones_col = cp.tile([128, 1], F32)
            nc.any.memset(ones_col[:], 1.0)
            ones_row = cp.tile([1, 128], F32)
            nc.any.memset(ones_row[:], 1.0)

            # --- tiny setup matmuls: U = filt^T @ wloc, pq^T = wq^T @ q^T ---
            with tc.tile_pool(name="setup_ps", bufs=1, space="PSUM") as sps:
                u_ps = sps.tile([F, A], F32)
                nc.tensor.matmul(u_ps[:], filt_sb[:], wloc_sb[:], start=True, stop=True)
                u_sb = cp.tile([F, A], BF16)
                nc.scalar.copy(u_sb[:], u_ps[:])

                pq_ps = sps.tile([A, BL], F32)
                for hk in range(HK):
                    nc.tensor.matmul(
                        pq_ps[:], wq_sb[:, hk], qT_sb[:, hk],
                        start=(hk == 0), stop=(hk == HK - 1),
                    )
                pq_sb = cp.tile([A, BL], F32)
                nc.scalar.copy(pq_sb[:], pq_ps[:])

            score_pool = ctx.enter_context(
                tc.tile_pool(name="score_ps", bufs=1, space="PSUM")
            )
            score_ps = score_pool.tile([128, BL * TH * NB], F32)

            # --- pass 1: scores ---
            p1 = ctx.enter_context(tc.tile_pool(name="p1", bufs=3))
            p1ps = ctx.enter_context(tc.tile_pool(name="p1ps", bufs=2, space="PSUM"))
            for b in range(BL):
                for th in range(TH):
                    t0 = th * 512
                    enct = p1.tile([128, HK, 512], BF16, tag="enct")
                    nc.sync.dma_start(
                        enct[:],
                        encT[b].rearrange("(hk p) t -> p hk t", p=128)[:, :, t0:t0 + 512],
                    )
                    s_ps = p1ps.tile([A, 512], F32, tag="s")
                    for hk in range(HK):
                        nc.tensor.matmul(
                            s_ps[:], wk_sb[:, hk], enct[:, hk],
                            start=(hk == 0), stop=False,
                        )
                    off = b * ROW + t0
                    nc.tensor.matmul(
                        s_ps[:], u_sb[:], pa_rep[:, off:off + 512],
                        start=False, stop=True,
                    )
                    tanh_t = p1.tile([A, 512], BF16, tag="tanh")
                    nc.scalar.activation(
                        tanh_t[:], s_ps[:], AF.Tanh, bias=pq_sb[:, b:b + 1]
                    )
                    for c in range(NB):
                        gc = (b * TH + th) * NB + c
                        nc.tensor.matmul(
                            score_ps[:, gc:gc + 1],
                            tanh_t[:, c * 128:(c + 1) * 128],
                            vv_sb[:],
                            start=True, stop=True,
                        )

            # --- softmax over tokens (no max-subtract: scores are O(1)) ---
            expsc = cp.tile([128, 128], F32)
            nc.scalar.activation(expsc[:], score_ps[:], AF.Exp)
            with tc.tile_pool(name="soft_ps", bufs=1, space="PSUM") as zps_pool:
                z_ps = zps_pool.tile([1, 128], F32)
                nc.tensor.matmul(z_ps[:], ones_col[:], expsc[:], start=True, stop=True)
                z_sb = cp.tile([1, 128], F32)
                nc.vector.tensor_copy(z_sb[:], z_ps[:])
                zb = cp.tile([1, BL], F32)
                nc.vector.reduce_sum(
                    zb[:], z_sb[:].rearrange("p (b e) -> p b e", e=TH * NB),
                    axis=mybir.AxisListType.X,
                )
                rz = cp.tile([1, BL], F32)
                nc.vector.reciprocal(rz[:], zb[:])
                rzrow = cp.tile([1, 128], F32)
                nc.vector.tensor_copy(
                    rzrow[:].rearrange("p (b e) -> p b e", e=TH * NB),
                    rz[:].unsqueeze(2).broadcast_to([1, BL, TH * NB]),
                )
                rz_ps = zps_pool.tile([128, 128], F32)
                nc.tensor.matmul(rz_ps[:], ones_row[:], rzrow[:], start=True, stop=True)
                al_f32 = cp.tile([128, 128], F32)
                nc.vector.tensor_mul(al_f32[:], expsc[:], rz_ps[:])
            al_bf = cp.tile([128, 128], BF16)
            nc.vector.tensor_copy(al_bf[:], al_f32[:])

            # alignment out: [p, (b th c)] -> [t, b] with t = th*512 + c*128 + p
            # (DMA APs are limited to 3 dims incl partition -> one DMA per
            # 128-token chunk: [128 p, 16 b] each)
            al_src = al_f32[:].rearrange("p (b tc) -> tc p b", tc=TH * NB)
            al_dst = al_out.rearrange("(tc p) b -> tc p b", p=128)
            for tc_i in range(TH * NB):
                nc.sync.dma_start(al_dst[tc_i], al_src[tc_i])

            # --- pass 2: context ---
            p2 = ctx.enter_context(tc.tile_pool(name="p2", bufs=3))
            p2ps = ctx.enter_context(tc.tile_pool(name="p2ps", bufs=2, space="PSUM"))
            ctx_sb = cp.tile([1, BL * H], F32)
            for b in range(BL):
                c_ps = p2ps.tile([1, H], F32, tag="ctx")
                for th in range(TH):
                    t0 = th * 512
                    encn = p2.tile([128, NB, H], BF16, tag="encn")
                    nc.sync.dma_start(
                        encn[:],
                        encN[b, t0:t0 + 512].rearrange("(k p) h -> p k h", p=128),
                    )
                    for k in range(NB):
                        gc = (b * TH + th) * NB + k
                        nc.tensor.matmul(
                            c_ps[:], al_bf[:, gc:gc + 1], encn[:, k],
                            start=(th == 0 and k == 0),
                            stop=(th == TH - 1 and k == NB - 1),
                        )
                nc.scalar.copy(ctx_sb[:, b * H:(b + 1) * H], c_ps[:])
            nc.sync.dma_start(ctx_out.rearrange("b h -> (b h)").unsqueeze(0), ctx_sb[:])

    nc.compile()
    return nc


_PROGRAM_CACHE: list = []


def _get_program() -> bacc.Bacc:
    if not _PROGRAM_CACHE:
        _PROGRAM_CACHE.append(_build_program())
    return _PROGRAM_CACHE[0]


def _host_shard(encoded_tokens, query, previous_alignment, conv_filter, W_loc, W_q, W_k, v):
    bf16 = ml_dtypes.bfloat16
    enc_bt = np.ascontiguousarray(encoded_tokens.transpose(1, 0, 2))  # [B, T, H]
    encN_all = enc_bt.astype(bf16)                                    # [B, T, H]
    encT_all = np.ascontiguousarray(enc_bt.transpose(0, 2, 1)).astype(bf16)  # [B, H, T]

    wk_h = W_k.astype(bf16)
    v_h = v.reshape(A, 1).astype(bf16)
    filt_h = np.ascontiguousarray(conv_filter.reshape(K, F)).astype(np.float32)
    wloc_h = W_loc.astype(np.float32)
    wq_h = W_q.astype(np.float32)

    in_maps = []
    for c in range(NCORES):
        b0 = c * BL
        pa_pad = np.zeros((BL, ROW), np.float32)
        pa_pad[:, PAD:PAD + T] = previous_alignment[:, b0:b0 + BL].T
        pa_flat = np.zeros(PA_LEN, np.float32)
        pa_flat[:BL * ROW] = pa_pad.reshape(-1)
        in_maps.append({
            "encT": encT_all[b0:b0 + BL],
            "encN": encN_all[b0:b0 + BL],
            "paT": pa_flat.astype(bf16),
            "qT": np.ascontiguousarray(query[b0:b0 + BL].T).astype(np.float32),
            "wk": wk_h,
            "wq": wq_h,
            "wloc": wloc_h,
            "filt": filt_h,
            "vv": v_h,
        })
    return in_maps


def run(inputs: dict, trace: bool = False):
    nc = _get_program()
    in_maps = _host_shard(**inputs)
    res = run_bass_kernel_spmd(nc, in_maps, list(range(NCORES)), trace=trace)
    context = np.concatenate([res.results[c]["ctx_out"] for c in range(NCORES)], axis=0)
    alignment = np.concatenate(
        [res.results[c]["al_out"] for c in range(NCORES)], axis=1
    )
    return (context.astype(np.float32), alignment.astype(np.float32)), res


def kernel(**inputs):
    (context, alignment), _ = run(inputs, trace=False)
    return context, alignment
